# revision 1
# baseline (speedup 1.0000x reference)
"""Trainium2 Bass kernel for nn_AttentionHead_Hybrid2 (B=4, N=4096, DK=64).

reference:
    V = x @ Wv.T + bv              (B,N,DK)
    Q = x @ wq ; K = x @ wk        (B,N)
    A = exp(-(Q_i - K_j)^2)        (B,N,N)
    P = softmax(A / 8, axis=-1)
    out = LN(P @ V + x)

Sharding: 8 cores = (batch b = c//2) x (query half c%2). Each core gets the
full key/value set for its batch (rolled so its 2048 queries are rows 0:2048)
and produces its 2048x64 output slice.

Key idea: the score between query i and key j depends on j ONLY through the
scalar K_j. Keys are binned onto a uniform M-point grid over K-space with
linear (hat-function) interpolation, which is exact to O(delta^2) and whose
per-key errors oscillate in sign and wash out over 4096 keys:
    e(Q_i,K_j) ~= sum_m w_jm e(Q_i,kappa_m),  w_jm = hat((K_j-kappa_m)/delta)
so, with xa = [x | 1]:
    out_i = sum_j e_ij [V|1]_j = sum_m E(Q_i,kappa_m) * bva_m
    bva   = (W.T @ xa) @ [Wv.T|..; bv|..]         (bin-major "binned V")
collapsing the (2048 x 4096) score work to (2048 x M) plus cheap binning.

Precision strategy: every PE matmul runs in bf16 (4x the fp32 rate), with
hi/lo splitting wherever bf16 quantization would show: x is shipped as
bf16 hi + lo residual pairs (natural and transposed), E is shipped to the
PE as E-1 in bf16 (the +1 folds into the softmax ratio via the exact bin
column-sums), and binned-V is split hi/lo on chip. The hat weight is
w = 1 - min(|iota' + n_j|, 1) (one ACT Abs + one DVE min per key tile,
scale/offset/clamp baked into host constants); the "1 -" folds out of the
binning matmul through the G ones-column, cancelling exactly.

Phases: prep (K cols, Q row, q_rep outer) -> bin (G += xa.T @ minU) ->
score (E chunks: Square/Exp/Exp + accT += bva.T @ E') -> finish (transpose,
ratio, +x, LayerNorm), finish pipelined in two query halves.
"""

import sys

for _p in ("/opt/trn_rl_repo", "/root/.axon_site/_ro/trn_rl_repo"):
    if _p not in sys.path:
        sys.path.insert(0, _p)

import numpy as np

import concourse.bass as bass
import concourse.mybir as mybir
import concourse.tile as tile
import bass_rust
from concourse.bass_utils import run_bass_kernel_spmd

F32 = mybir.dt.float32
BF16 = mybir.dt.bfloat16
I32 = mybir.dt.int32
AF = mybir.ActivationFunctionType
OP = mybir.AluOpType

B, N, DK = 4, 4096, 64
NQ = 2048          # queries per core
NCORES = 8
JT = N // 128      # 32 key tiles
IT = NQ // 128     # 16 query tiles
M = 256            # K-grid bins
MC = M // 128      # bin chunks
K0 = -5.5
DELTA = 11.0 / (M - 1)
EPS = 1e-5

# packed const blob layout (128 partitions wide)
_IDENT0 = 0
_KAP0 = 128
_GAM0 = _KAP0 + MC
_BET0 = _GAM0 + DK
BLOB_W = _BET0 + DK


def split_multiwaits(nc):
    """Walrus in this env accepts one sem-wait per instruction; Tile emits
    several. Split extras onto preceding same-engine NoOps."""
    ctr = 0
    for f in nc.m.functions:
        for bb in f.blocks:
            out, changed = [], False
            for ins in bb.instructions:
                si = ins.sync_info
                if si is not None and si.on_wait and len(si.on_wait) > 1:
                    waits = list(si.on_wait)
                    for w in waits[:-1]:
                        ctr += 1
                        out.append(mybir.InstNoOp(
                            name=f"I-wsplit-{ctr}", engine=ins.engine,
                            debug=ins.debug, ins=[], outs=[],
                            sync_info=bass_rust.SyncInfo(on_wait=[w], on_update=[])))
                    ins.sync_info = bass_rust.SyncInfo(
                        on_wait=[waits[-1]], on_update=list(si.on_update or []))
                    changed = True
                out.append(ins)
            if changed:
                bb.instructions = out
    return ctr


def build_nc(split=True):
    nc = bass.Bass("TRN2", target_bir_lowering=False, debug=False)

    xa_d = nc.dram_tensor("xa", [N, 65], BF16, kind="ExternalInput").ap()
    xl_d = nc.dram_tensor("xl", [NQ, 65], BF16, kind="ExternalInput").ap()
    xth_d = nc.dram_tensor("xth", [DK, N], BF16, kind="ExternalInput").ap()
    xtl_d = nc.dram_tensor("xtl", [DK, N], BF16, kind="ExternalInput").ap()
    wvkb_d = nc.dram_tensor("wvkb", [DK + 1, 67], F32, kind="ExternalInput").ap()
    blob_d = nc.dram_tensor("blob", [128, BLOB_W], F32, kind="ExternalInput").ap()
    out_d = nc.dram_tensor("out", [NQ, DK], F32, kind="ExternalOutput").ap()

    with tile.TileContext(nc) as tc:
        cpool = tc.alloc_tile_pool(name="consts", bufs=1)
        big = tc.alloc_tile_pool(name="big", bufs=1)

        wvkb = cpool.tile([DK + 1, 67], F32)
        nc.sync.dma_start(wvkb[:], wvkb_d[:])
        wvkh = cpool.tile([DK + 1, 67], BF16)
        nc.vector.tensor_copy(wvkh[:], wvkb[:])
        wvkl = cpool.tile([DK + 1, 67], BF16)
        nc.vector.tensor_tensor(wvkl[:], wvkb[:], wvkh[:], OP.subtract)

        xth = big.tile([DK, N], BF16)
        xtl = big.tile([DK, N], BF16)

        blob = cpool.tile([128, BLOB_W], F32)
        ident = blob[:, _IDENT0:_IDENT0 + 128]
        kap = blob[:, _KAP0:_KAP0 + MC]
        gam = blob[:, _GAM0:_GAM0 + DK]
        bet = blob[:, _BET0:_BET0 + DK]

        eps_c = cpool.tile([128, 2], F32)
        nc.gpsimd.memset(eps_c[:], EPS)
        nc.scalar.activation(eps_c[:, 1:2], eps_c[:, 0:1], AF.Abs, scale=1.0)
        ones_f = cpool.tile([1, 128], F32)
        nc.gpsimd.memset(ones_f[:], 1.0)
        # iota' = m + K0/DELTA, built on device; cols M, M+1 are sentinels that
        # force hat weight 0 there, making the matmul's last columns pure
        # ones-columns (w = 1 - min(|big|,1) ... min saturates to 1? no: the
        # sentinel forces u>=1 so min=1; those columns carry sum(xa*1) after
        # the 1-complement, i.e. they become the exact xsum columns)
        ioti = cpool.tile([128, M + 2], I32)
        nc.gpsimd.iota(ioti[:], [[1, M + 2]], channel_multiplier=0)
        iota = cpool.tile([128, M + 2], F32)
        nc.vector.tensor_scalar(iota[:], ioti[:], 1.0, K0 / DELTA, OP.mult, OP.add)
        nc.gpsimd.memset(iota[:, M:M + 2], 1.0e6)

        xa_all = big.tile([128, JT * 65], BF16)      # [x | 1] natural, bf16 hi
        xa_v = xa_all.rearrange("p (t c) -> p t c", c=65)
        xl_all = big.tile([128, IT * 65], BF16)      # lo residual (queries)
        xl_v = xl_all.rearrange("p (t c) -> p t c", c=65)
        for h in range(4):
            nc.gpsimd.dma_start(
                xa_v[:, h * 8:(h + 1) * 8, :],
                xa_d[h * 1024:(h + 1) * 1024, :].rearrange("(t p) c -> p t c", p=128))
            if h < 2:
                nc.gpsimd.dma_start(
                    xl_v[:, h * 8:(h + 1) * 8, :],
                    xl_d[h * 1024:(h + 1) * 1024, :].rearrange("(t p) c -> p t c",
                                                               p=128))


        tcol = big.tile([128, JT], F32)              # hat bias -clamp(K/d ...)
        q_sb = big.tile([1, NQ], F32)
        q_rep = big.tile([128, NQ], F32)             # Q replicated across partitions
        ep_full = big.tile([128, MC * NQ], BF16)     # E-1 per bin chunk (bf16)
        ep_v = ep_full.rearrange("p (t i) -> p t i", i=NQ)

        with tc.tile_pool(name="prep_ps", bufs=2, space="PSUM") as pps:
            for h in range(8):
                c0, c1 = h * 512, (h + 1) * 512
                nc.sync.dma_start(xth[:, c0:c1], xth_d[:, c0:c1])
                if h < 4:
                    nc.sync.dma_start(xtl[:, c0:c1], xtl_d[:, c0:c1])
                for jt in range(h * 4, (h + 1) * 4):
                    kc = pps.tile([128, 1], F32, tag="kc")
                    nc.tensor.matmul(kc[:], xth[:, jt * 128:(jt + 1) * 128],
                                     wvkh[0:DK, 65:66], start=True, stop=True)
                    # bias n = -clamp(K/d, K0/d, K0/d + M-1); kc holds -K/d
                    nc.vector.tensor_scalar(tcol[:, jt:jt + 1], kc[:],
                                            -K0 / DELTA - (M - 1), -K0 / DELTA,
                                            OP.max, OP.min)
                if h >= 4:
                    continue
                # Q row chunk (hi+lo accumulated)
                qp = pps.tile([1, 512], F32, tag="qp")
                nc.tensor.matmul(qp[:], wvkh[0:DK, 66:67], xth[:, c0:c1],
                                 start=True, stop=False)
                nc.tensor.matmul(qp[:], wvkl[0:DK, 66:67], xtl[:, c0:c1],
                                 start=False, stop=True)
                nc.vector.tensor_copy(q_sb[0:1, c0:c1], qp[:])
            # blob (kap/ident/gamma/beta): queued behind the critical chunks
            nc.sync.dma_start(blob[:], blob_d[:])
            # replicate Q across partitions via fp32 ones-outer
            for ic in range(NQ // 512):
                qr = pps.tile([128, 512], F32, tag="qr")
                nc.tensor.matmul(qr[:], ones_f[0:1, :],
                                 q_sb[0:1, ic * 512:(ic + 1) * 512],
                                 start=True, stop=True)
                nc.vector.tensor_copy(q_rep[:, ic * 512:(ic + 1) * 512], qr[:])


        def emit_e_chunk(mc, ep_):
            sq = ep_.tile([128, NQ], F32, tag="sq")
            nc.scalar.activation(sq[:], q_rep[:], AF.Square,
                                 bias=kap[:, mc:mc + 1], scale=-1.0)
            a_t = ep_.tile([128, NQ], F32, tag="a")
            nc.scalar.activation(a_t[:], sq[:], AF.Exp, scale=-1.0)
            e_t = ep_.tile([128, NQ], F32, tag="e")
            nc.scalar.activation(e_t[:], a_t[:], AF.Exp, scale=0.125)
            # E' = E - 1 in bf16 for the PE (the +1 rides the bin colsums)
            if mc % 2 == 0:
                nc.vector.tensor_scalar(ep_v[:, mc, :], e_t[:], -1.0, None, OP.add)
            else:
                nc.scalar.activation(ep_v[:, mc, :], e_t[:], AF.Copy, bias=-1.0)

        # ---- binning + interleaved score passes ----
        H = big.tile([DK + 1, M], F32)
        with tc.tile_pool(name="g_ps", bufs=1, space="PSUM") as gp:
            G = gp.tile([DK + 1, M + 2], F32)        # cols M,M+1 = min-ones
            with (tc.tile_pool(name="w_sb", bufs=8) as wp,
                  tc.tile_pool(name="e_scr", bufs=3) as ep_):
                for jt in range(JT):
                    u_t = wp.tile([128, M + 2], BF16, tag="u")
                    nc.scalar.activation(u_t[:], iota[:], AF.Abs,
                                         bias=tcol[:, jt:jt + 1], scale=1.0)
                    w_t = wp.tile([128, M + 2], BF16, tag="w")
                    nc.vector.tensor_scalar(w_t[:], u_t[:], 1.0, None, OP.min)
                    nc.tensor.matmul(G[:], xa_v[:, jt, :], w_t[:],
                                     start=(jt == 0), stop=(jt == JT - 1))
                    if jt in (17, 25):
                        emit_e_chunk((jt - 17) // 8, ep_)
            # col M of G now holds sum_j min(big,1)*xa = xsum exactly
            G_sb = big.tile([DK + 1, M + 1], F32)
            nc.vector.tensor_copy(G_sb[:], G[:, 0:M + 1])
            # hat = 1 - min(u,1) => binned xa = xsum - G, far terms cancel
            nc.vector.tensor_tensor(H[:], G_sb[:, M:M + 1].broadcast_to([DK + 1, M]),
                                    G_sb[:, 0:M], OP.subtract)

        # bva = H.T @ wvkb  (bin-major binned [V|count]), split hi/lo bf16
        bvah = big.tile([128, MC * 65], BF16)
        bvah_v = bvah.rearrange("p (t c) -> p t c", c=65)
        bval = big.tile([128, MC * 65], BF16)
        bval_v = bval.rearrange("p (t c) -> p t c", c=65)
        colsum = big.tile([65, 1], F32)
        with tc.tile_pool(name="bva_ps", bufs=3, space="PSUM") as bp:
            for mc in range(MC):
                bt = bp.tile([128, 65], F32, tag="bt")
                nc.tensor.matmul(bt[:], H[:, mc * 128:(mc + 1) * 128],
                                 wvkb[:, 0:65], start=True, stop=True)
                nc.vector.tensor_copy(bvah_v[:, mc, :], bt[:])
                nc.vector.tensor_tensor(bval_v[:, mc, :], bt[:], bvah_v[:, mc, :],
                                        OP.subtract)
            # colsum of Vaug = G ones-column pushed through the weights
            cs = bp.tile([65, 1], F32, tag="cs")
            nc.tensor.matmul(cs[:], wvkb[:, 0:65], G_sb[0:DK + 1, M:M + 1],
                             start=True, stop=True)
            nc.vector.tensor_copy(colsum[:], cs[:])

        # ---- score matmuls + pipelined finish (two query halves) ----
        with tc.tile_pool(name="acc_ps", bufs=4, space="PSUM") as accp:
            outT = big.tile([65, NQ], F32)
            nat = big.tile([128, IT * 65], F32)
            nat_v = nat.rearrange("p (t c) -> p t c", c=65)
            fin = big.tile([128, IT * DK], F32)
            fin_v = fin.rearrange("p (t d) -> p t d", d=DK)
            xq = big.tile([128, IT * DK], F32)
            xq_v = xq.rearrange("p (t d) -> p t d", d=DK)
            nc.vector.tensor_tensor(xq_v[:], xa_v[:, 0:IT, 0:DK],
                                    xl_v[:, 0:IT, 0:DK], OP.add)
            rec = big.tile([128, IT], F32)
            stat = big.tile([128, 4 * IT], F32)
            scr = big.tile([128, IT * DK], F32)
            scr_v = scr.rearrange("p (t d) -> p t d", d=DK)

            NH = 4                                   # finish pipeline chunks
            HT = IT // NH
            with tc.tile_pool(name="fin_ps", bufs=4, space="PSUM") as finp:
                for h in range(NH):
                    i0, i1 = h * (NQ // NH), (h + 1) * (NQ // NH)
                    accT = accp.tile([65, NQ // NH], F32, tag="acc")
                    for mc in range(MC):
                        nc.tensor.matmul(accT[:], bvah_v[:, mc, :],
                                         ep_v[:, mc, i0:i1],
                                         start=(mc == 0), stop=False)
                        nc.tensor.matmul(accT[:], bval_v[:, mc, :],
                                         ep_v[:, mc, i0:i1],
                                         start=False, stop=(mc == MC - 1))
                    # outT = accT + colsum  (restores the +1 of E = 1 + E')
                    nc.vector.tensor_tensor(
                        outT[:, i0:i1], accT[:],
                        colsum.broadcast_to([65, NQ // NH]), OP.add)
                    np_t = finp.tile([128, HT * 65], F32, tag="nat")
                    for q2 in range(HT):
                        it = h * HT + q2
                        nc.tensor.transpose(np_t[:, q2 * 65:(q2 + 1) * 65],
                                            outT[:, it * 128:(it + 1) * 128],
                                            ident[0:65, 0:65])
                    nc.vector.tensor_copy(nat_v[:, h * HT:(h + 1) * HT, :], np_t[:])

                    ts_, te_ = h * HT, (h + 1) * HT
                    n_v = nat_v[:, ts_:te_, :]
                    f_v = fin_v[:, ts_:te_, :]
                    s_v = scr_v[:, ts_:te_, :]
                    sum_ = stat[:, 0 * IT + ts_:0 * IT + te_]
                    m_ = stat[:, 1 * IT + ts_:1 * IT + te_]
                    v_ = stat[:, 2 * IT + ts_:2 * IT + te_]
                    rstd = stat[:, 3 * IT + ts_:3 * IT + te_]
                    rc = rec[:, ts_:te_]

                    nc.vector.reciprocal(rc, n_v[:, :, 64])
                    nc.vector.tensor_tensor(
                        f_v, n_v[:, :, 0:DK],
                        rc.unsqueeze(-1).broadcast_to([128, HT, DK]), OP.mult)
                    nc.vector.tensor_tensor(f_v, f_v, xq_v[:, ts_:te_, :], OP.add)
                    nc.vector.reduce_sum(sum_, f_v, axis=mybir.AxisListType.X)
                    nc.vector.tensor_scalar_mul(m_, sum_, 1.0 / DK)
                    nc.vector.tensor_tensor(
                        f_v, f_v, m_.unsqueeze(-1).broadcast_to([128, HT, DK]),
                        OP.subtract)
                    nc.scalar.activation(s_v.rearrange("p t d -> p (t d)"),
                                         f_v.rearrange("p t d -> p (t d)"),
                                         AF.Square, scale=1.0)
                    nc.vector.reduce_sum(v_, s_v, axis=mybir.AxisListType.X)
                    nc.scalar.activation(rstd, v_, AF.Ln, bias=eps_c[:, 0:1],
                                         scale=1.0 / DK)
                    nc.scalar.activation(rstd, rstd, AF.Exp, scale=-0.5)
                    nc.vector.tensor_tensor(
                        f_v, f_v, rstd.unsqueeze(-1).broadcast_to([128, HT, DK]),
                        OP.mult)
                    nc.vector.tensor_tensor(
                        f_v, f_v, gam.unsqueeze(1).broadcast_to([128, HT, DK]),
                        OP.mult)
                    nc.vector.tensor_tensor(
                        f_v, f_v, bet.unsqueeze(1).broadcast_to([128, HT, DK]),
                        OP.add)
                    nc.sync.dma_start(
                        out_d[i0:i1, :].rearrange("(t p) d -> p t d", p=128), f_v)

        big.release()
        cpool.release()

    if split:
        split_multiwaits(nc)
    return nc


_NC_CACHE = None


def _get_nc():
    global _NC_CACHE
    if _NC_CACHE is None:
        _NC_CACHE = build_nc()
    return _NC_CACHE


def make_in_maps(x, Wv, bv, wq, wk, gamma, beta):
    import ml_dtypes
    x = np.asarray(x, np.float32)
    wkp = (np.asarray(wk, np.float64) * (-1.0 / DELTA)).astype(np.float32)
    wvk = np.concatenate([np.asarray(Wv, np.float32).T,
                          np.zeros((DK, 1), np.float32),
                          wkp[:, None],
                          np.asarray(wq, np.float32)[:, None]], axis=1)
    brow = np.concatenate([np.asarray(bv, np.float32), [1.0, 0.0, 0.0]]
                          ).astype(np.float32)
    wvkb = np.concatenate([wvk, brow[None, :]], axis=0).copy()      # (65, 67)

    blob = np.zeros((128, BLOB_W), np.float32)
    blob[:, _IDENT0:_IDENT0 + 128] = np.eye(128, dtype=np.float32)
    kgrid = (K0 + DELTA * np.arange(M, dtype=np.float64)).astype(np.float32)
    blob[:, _KAP0:_KAP0 + MC] = kgrid.reshape(MC, 128).T
    blob[:, _GAM0:_GAM0 + DK] = np.asarray(gamma, np.float32)[None, :]
    blob[:, _BET0:_BET0 + DK] = np.asarray(beta, np.float32)[None, :]

    ones = np.ones((N, 1), np.float32)
    in_maps = []
    for c in range(NCORES):
        b, qoff = c // 2, (c % 2) * NQ
        xr = np.concatenate([x[b, qoff:], x[b, :qoff]], axis=0) if qoff else x[b]
        xaf = np.concatenate([xr, ones], 1)
        xa = xaf.astype(ml_dtypes.bfloat16)
        xl = (xaf[0:NQ] - xa[0:NQ].astype(np.float32)).astype(ml_dtypes.bfloat16)
        xtf = np.ascontiguousarray(xr.T)
        xth = xtf.astype(ml_dtypes.bfloat16)
        xtl = (xtf - xth.astype(np.float32)).astype(ml_dtypes.bfloat16)
        in_maps.append({"xa": np.ascontiguousarray(xa),
                        "xl": np.ascontiguousarray(xl),
                        "xth": np.ascontiguousarray(xth),
                        "xtl": np.ascontiguousarray(xtl),
                        "wvkb": wvkb, "blob": blob})
    return in_maps


def kernel(x, Wv, bv, wq, wk, gamma, beta, _trace=False, _trace_cores=None):
    nc = _get_nc()
    in_maps = make_in_maps(x, Wv, bv, wq, wk, gamma, beta)
    res = run_bass_kernel_spmd(nc, in_maps, core_ids=list(range(NCORES)),
                               trace=_trace, trace_cores=_trace_cores)
    out = np.empty((B, N, DK), np.float32)
    for c in range(NCORES):
        b, qoff = c // 2, (c % 2) * NQ
        out[b, qoff:qoff + NQ] = res.results[c]["out"]
    kernel._last_results = res
    return out



# revision 5
# speedup vs baseline: 1.3191x; 1.3191x over previous
"""Trainium2 Bass kernel for nn_AttentionHead_Hybrid2 (B=4, N=4096, DK=64).

reference:
    V = x @ Wv.T + bv              (B,N,DK)
    Q = x @ wq ; K = x @ wk        (B,N)
    A = exp(-(Q_i - K_j)^2)        (B,N,N)
    P = softmax(A / 8, axis=-1)
    out = LN(P @ V + x)

Sharding: 8 cores = (batch b = c//2) x (query half c%2). Each core gets the
full key set for its batch (rolled so its 2048 queries are rows 0:2048) and
produces its 2048x64 output slice.

Algorithm (Fourier separation): the score E(q,k) = exp(exp(-(q-k)^2)/8)
depends only on t = q - k, so it has a rapidly-converging cosine expansion
E(t) = sum_k a_k cos(w_k t) (periodized, L=13, 24 cos/sin features gives
~3e-5 abs accuracy). cos(w(Q-K)) = cosQcosK + sinQsinK makes attention
separable with rank 24:
    num (2048, 66) = PhiQ (2048,24) @ [ a*(Wv-transformed PhiK-moments) ]
where PhiK/PhiQ are sin/cos feature maps of the key/query scalar
projections, and the Wv/bv/ones columns ride along (col 64 = softmax
denominator, col 65 = row-sum of the numerator for LN stats).

No (N,N) score matrix, no per-key-tile hat binning, no big exp fields:
the per-token transcendental work is 24 sin values per token, evaluated by
the PE (phase outer products, in turns r = u/2pi), one DVE magic-number
round pass (the ACT sin table is only valid on [-pi,pi]), and one ACT Sin
pass sin(2pi*w) = sin(-2pi*b), w = r - round(r).

LayerNorm is scale-invariant, so no division by the softmax denominator:
z = num + den*x, out = (z - mean(z)) * rsqrt(var(z)); gamma/beta are
ones/zeros per the problem spec (host applies them if they ever are not).

Phases: DMA (xth1 fp32 transposed+ones-row, xa/xl bf16 natural) ->
phase outers (48 rank-65 matmuls) -> round/sub (DVE) -> Sin (ACT) ->
[keys: F-accum matmuls; queries: PE transposes -> Sin rotated] ->
Fw = F.T@WVB * a (hi/lo bf16) -> per-tile numerator matmuls (tokens on
partitions, no output transpose) -> fused LN tail (uncentered moments,
batched Ln/Exp so the ACT table switches exactly once).
"""

import math
import sys

for _p in ("/opt/trn_rl_repo", "/root/.axon_site/_ro/trn_rl_repo"):
    if _p not in sys.path:
        sys.path.insert(0, _p)

import numpy as np

import concourse.bass as bass
import concourse.mybir as mybir
import concourse.tile as tile
import bass_rust
from concourse.bass_utils import run_bass_kernel_spmd

F32 = mybir.dt.float32
BF16 = mybir.dt.bfloat16
AF = mybir.ActivationFunctionType
OP = mybir.AluOpType

B, N, DK = 4, 4096, 64
NQ = 2048          # queries per core
NCORES = 8
JT = N // 128      # 32 key tiles
IT = NQ // 128     # 16 query tiles
LPER = 13.0        # Fourier period in t = q - k
NF = 24            # features: cos k=0..12, sin k=1..11
MAGIC = float(np.float32(1.5 * 2 ** 23))   # fp32 round-to-nearest trick
GRP = 16           # phase tiles per round/sin group (16*24=384 cols/bank)

# const blob column layout (128 partitions x BLOB_W fp32)
_WVB0 = 0                 # (65, 66)
_W2K0 = _WVB0 + 66        # (65, 24)
_W2Q0 = _W2K0 + 24        # (65, 24)
_ACOL = _W2Q0 + 24        # (24, 1)  feature coefficients
_IDEN = _ACOL + 1         # (128, 128) identity (transposes)
BLOB_W = _IDEN + 128


def split_multiwaits(nc):
    """Walrus in this env accepts one sem-wait per instruction; Tile emits
    several. Split extras onto preceding same-engine NoOps."""
    ctr = 0
    for f in nc.m.functions:
        for bb in f.blocks:
            out, changed = [], False
            for ins in bb.instructions:
                si = ins.sync_info
                if si is not None and si.on_wait and len(si.on_wait) > 1:
                    waits = list(si.on_wait)
                    for w in waits[:-1]:
                        ctr += 1
                        out.append(mybir.InstNoOp(
                            name=f"I-wsplit-{ctr}", engine=ins.engine,
                            debug=ins.debug, ins=[], outs=[],
                            sync_info=bass_rust.SyncInfo(on_wait=[w], on_update=[])))
                    ins.sync_info = bass_rust.SyncInfo(
                        on_wait=[waits[-1]], on_update=list(si.on_update or []))
                    changed = True
                out.append(ins)
            if changed:
                bb.instructions = out
    return ctr


def build_nc(split=True):
    nc = bass.Bass("TRN2", target_bir_lowering=False, debug=False)

    xth1_d = nc.dram_tensor("xth1", [DK + 1, N], F32, kind="ExternalInput").ap()
    xa_d = nc.dram_tensor("xa", [N, 65], BF16, kind="ExternalInput").ap()
    xl_d = nc.dram_tensor("xl", [NQ, DK], BF16, kind="ExternalInput").ap()
    blob_d = nc.dram_tensor("blob", [128, BLOB_W], F32, kind="ExternalInput").ap()
    out_d = nc.dram_tensor("out", [NQ, DK], F32, kind="ExternalOutput").ap()

    with tile.TileContext(nc) as tc:
        cpool = tc.alloc_tile_pool(name="consts", bufs=1)
        big = tc.alloc_tile_pool(name="big", bufs=1)

        blob = cpool.tile([128, BLOB_W], F32)
        nc.sync.dma_start(blob[:], blob_d[:])
        wvb = blob[0:65, _WVB0:_WVB0 + 66]
        w2k = blob[0:65, _W2K0:_W2K0 + 24]
        w2q = blob[0:65, _W2Q0:_W2Q0 + 24]
        acol = blob[0:24, _ACOL:_ACOL + 1]
        ident = blob[:, _IDEN:_IDEN + 128]

        xth1 = big.tile([DK + 1, N], F32)
        for h in range(8):
            nc.sync.dma_start(xth1[:, h * 512:(h + 1) * 512],
                              xth1_d[:, h * 512:(h + 1) * 512])

        xa_all = big.tile([128, JT * 65], BF16)
        xa_v = xa_all.rearrange("p (t c) -> p t c", c=65)
        xl_all = big.tile([128, IT * DK], BF16)
        xl_v = xl_all.rearrange("p (t c) -> p t c", c=DK)
        for h in range(4):
            nc.gpsimd.dma_start(
                xa_v[:, h * 8:(h + 1) * 8, :],
                xa_d[h * 1024:(h + 1) * 1024, :].rearrange("(t p) c -> p t c", p=128))
            if h < 2:
                nc.gpsimd.dma_start(
                    xl_v[:, h * 8:(h + 1) * 8, :],
                    xl_d[h * 1024:(h + 1) * 1024, :].rearrange("(t p) c -> p t c",
                                                               p=128))

        # residual x (fp32) and per-token row sums, off the critical path
        xq = big.tile([128, IT * DK], F32)
        xq_v = xq.rearrange("p (t d) -> p t d", d=DK)
        xsum = big.tile([128, IT], F32)
        nc.vector.tensor_tensor(xq_v[:], xa_v[:, 0:IT, 0:DK], xl_v[:], OP.add)
        nc.vector.reduce_sum(xsum[:], xq_v[:], axis=mybir.AxisListType.X)

        # ---- phase features ----
        # groups: 0 = keys 0..15, 1 = keys 16..31, 2 = queries 0..15
        phk = big.tile([128, JT * NF], BF16)      # key features, tile-major
        phk_v = phk.rearrange("p (t f) -> p t f", f=NF)
        phq = big.tile([24, NQ], BF16)            # query features, rotated
        w_sb = big.tile([128, 3 * GRP * NF], F32)  # reduced phases (w = r - rt)
        w_v = w_sb.rearrange("p (g c) -> p g c", c=GRP * NF)

        with (tc.tile_pool(name="u_ps", bufs=3, space="PSUM") as ups,
              tc.tile_pool(name="wt_ps", bufs=4, space="PSUM") as wtps,
              tc.tile_pool(name="rt_sb", bufs=3) as rtp):
            wq_t = [None] * 4
            for g in range(3):
                u = ups.tile([128, GRP * NF], F32, tag="u")
                u_t = u.rearrange("p (t f) -> p t f", f=NF)
                w2 = w2q if g == 2 else w2k
                for i in range(GRP):
                    jt = i if g == 2 else g * GRP + i
                    nc.tensor.matmul(u_t[:, i, :],
                                     xth1[:, jt * 128:(jt + 1) * 128], w2,
                                     start=True, stop=True)
                rt = rtp.tile([128, GRP * NF], F32, tag="rt")
                nc.vector.tensor_scalar(rt[:], u[:], MAGIC, MAGIC, OP.add,
                                        OP.subtract)
                nc.vector.tensor_tensor(w_v[:, g, :], u[:], rt[:], OP.subtract)
                if g < 2:
                    # keys: sin in tile-major layout
                    nc.scalar.activation(
                        phk[:, g * GRP * NF:(g + 1) * GRP * NF],
                        w_v[:, g, :], AF.Sin, scale=2 * math.pi)
                else:
                    # queries: transpose w tiles to (24, NQ), then sin per bank
                    wg = w_v.rearrange("p g (t f) -> p g t f", f=NF)
                    for q in range(4):
                        wt = wtps.tile([24, 512], F32, tag="wt")
                        for i in range(4):
                            it = q * 4 + i
                            nc.tensor.transpose(
                                wt[:, i * 128:(i + 1) * 128],
                                wg[:, 2, it, :], ident)
                        wq_t[q] = wt
                    for q in range(4):
                        nc.scalar.activation(phq[:, q * 512:(q + 1) * 512],
                                             wq_t[q][:], AF.Sin,
                                             scale=2 * math.pi)

        # ---- key moments F and the transformed, coefficient-folded Fw ----
        fwh = big.tile([24, 66], BF16)
        fwl = big.tile([24, 66], BF16)
        with tc.tile_pool(name="f_ps", bufs=2, space="PSUM") as fps:
            f_ps = fps.tile([65, NF], F32, tag="f")
            for jt in range(JT):
                nc.tensor.matmul(f_ps[:], xa_v[:, jt, :], phk_v[:, jt, :],
                                 start=(jt == 0), stop=(jt == JT - 1))
            f_sb = big.tile([65, NF], F32)
            nc.vector.tensor_copy(f_sb[:], f_ps[:])
            fw_ps = fps.tile([24, 66], F32, tag="fw")
            nc.tensor.matmul(fw_ps[:], f_sb[:], wvb, start=True, stop=True)
            # fold coefficients a_k, split hi/lo bf16
            fwm = big.tile([24, 66], F32)
            nc.vector.tensor_tensor(fwm[:], fw_ps[:],
                                    acol.broadcast_to([24, 66]), OP.mult)
            nc.vector.tensor_copy(fwh[:], fwm[:])
            nc.vector.tensor_tensor(fwl[:], fwm[:], fwh[:], OP.subtract)

        # ---- numerator (tokens on partitions) + fused LN tail ----
        z_sb = big.tile([128, IT * DK], F32)
        z_v = z_sb.rearrange("p (t d) -> p t d", d=DK)
        o_sb = big.tile([128, IT * DK], F32)
        o_v = o_sb.rearrange("p (t d) -> p t d", d=DK)
        sq = big.tile([128, 8 * DK], F32)
        sq_v = sq.rearrange("p (t d) -> p t d", d=DK)
        st = big.tile([128, 8 * IT], F32)   # stats: s2, mu, var, rstd, -mu*rstd
        s2 = st[:, 0 * IT:1 * IT]
        mu_c = st[:, 2 * IT:3 * IT]
        var_c = st[:, 3 * IT:4 * IT]
        rstd_c = st[:, 4 * IT:5 * IT]
        nmu_c = st[:, 5 * IT:6 * IT]
        t1 = big.tile([128, 8 * DK], F32)
        t1_v = t1.rearrange("p (t d) -> p t d", d=DK)

        with tc.tile_pool(name="num_ps", bufs=1, space="PSUM") as nps:
            nf = nps.tile([128, IT * 128], F32)
            nf_v = nf.rearrange("p (t c) -> p t c", c=128)
            for it in range(IT):
                lhs = phq[:, it * 128:(it + 1) * 128]
                nc.tensor.matmul(nf_v[:, it, 0:66], lhs, fwh[:],
                                 start=True, stop=False)
                nc.tensor.matmul(nf_v[:, it, 0:66], lhs, fwl[:],
                                 start=False, stop=True)
            for h in range(4):
                ts_, te_ = h * 4, (h + 1) * 4
                sl = slice((h % 2) * 4, (h % 2) * 4 + 4)
                nv = nf_v[:, ts_:te_, :]
                mu3 = mu_c[:, ts_:te_].unsqueeze(-1)
                # z = num + den * x
                nc.vector.tensor_tensor(
                    t1_v[:, sl, :], xq_v[:, ts_:te_, :],
                    nv[:, :, 64:65].broadcast_to([128, 4, DK]), OP.mult)
                nc.vector.tensor_tensor(z_v[:, ts_:te_, :], t1_v[:, sl, :],
                                        nv[:, :, 0:64], OP.add)
                # sum z^2 (Square is in the trig table set: no table switch)
                nc.scalar.activation(sq_v[:, sl, :].rearrange("p t d -> p (t d)"),
                                     z_v[:, ts_:te_, :].rearrange("p t d -> p (t d)"),
                                     AF.Square, scale=1.0)
                nc.vector.reduce_sum(s2[:, ts_:te_], sq_v[:, sl, :],
                                     axis=mybir.AxisListType.X)
                # mu*64 = numsum + den*xsum
                nc.vector.tensor_tensor(mu3, nv[:, :, 64:65],
                                        xsum[:, ts_:te_].unsqueeze(-1), OP.mult)
                nc.vector.tensor_tensor(mu3, mu3, nv[:, :, 65:66], OP.add)
            # batched stats: mu, var = s2/64 - mu^2, rstd = exp(-.5 ln var)
            nc.vector.tensor_scalar_mul(mu_c[:], mu_c[:], 1.0 / DK)
            nc.vector.tensor_tensor(var_c[:], mu_c[:], mu_c[:], OP.mult)
            nc.vector.scalar_tensor_tensor(var_c[:], s2[:], 1.0 / DK, var_c[:],
                                           OP.mult, OP.subtract)
            nc.scalar.activation(rstd_c[:], var_c[:], AF.Ln, scale=1.0)
            nc.scalar.activation(rstd_c[:], rstd_c[:], AF.Exp, scale=-0.5)
            nc.vector.tensor_tensor(nmu_c[:], mu_c[:], rstd_c[:], OP.mult)
            nc.vector.tensor_scalar_mul(nmu_c[:], nmu_c[:], -1.0)
            for h in range(4):
                ts_, te_ = h * 4, (h + 1) * 4
                # out = z*rstd - mu*rstd
                nc.vector.tensor_tensor(
                    o_v[:, ts_:te_, :], z_v[:, ts_:te_, :],
                    rstd_c[:, ts_:te_].unsqueeze(-1).broadcast_to([128, 4, DK]),
                    OP.mult)
                nc.vector.tensor_tensor(
                    o_v[:, ts_:te_, :], o_v[:, ts_:te_, :],
                    nmu_c[:, ts_:te_].unsqueeze(-1).broadcast_to([128, 4, DK]),
                    OP.add)
                nc.sync.dma_start(
                    out_d[h * 512:(h + 1) * 512, :].rearrange(
                        "(t p) d -> p t d", p=128), o_v[:, ts_:te_, :])

        big.release()
        cpool.release()

    if split:
        split_multiwaits(nc)
    return nc


_NC_CACHE = None


def _get_nc():
    global _NC_CACHE
    if _NC_CACHE is None:
        _NC_CACHE = build_nc()
    return _NC_CACHE


def _fourier_coeffs():
    m = 16384
    t = LPER * np.arange(m) / m
    tw = np.minimum(t, LPER - t)
    g = np.exp(np.exp(-tw ** 2) / 8.0) - 1.0
    c = np.fft.rfft(g) / m
    a_cos = np.concatenate([[1.0 + np.real(c[0])], 2 * np.real(c[1:13])])
    a_sin = 2 * np.real(c[1:12])
    return np.concatenate([a_cos, a_sin]).astype(np.float32)


def make_in_maps(x, Wv, bv, wq, wk, gamma, beta):
    import ml_dtypes
    x = np.asarray(x, np.float32)
    kfeat = np.concatenate([np.arange(13), np.arange(1, 12)]).astype(np.float64)
    phip = np.concatenate([0.25 * np.ones(13), np.zeros(11)])
    afull = _fourier_coeffs()

    wvb = np.zeros((65, 66), np.float32)
    wvb[:64, :64] = np.asarray(Wv, np.float32).T
    wvb[64, :64] = np.asarray(bv, np.float32)
    wvb[64, 64] = 1.0
    wvb[:, 65] = wvb[:, :64].sum(1)

    def w2(w):
        return np.concatenate(
            [np.outer(np.asarray(w, np.float64), kfeat / LPER),
             phip[None, :]], 0).astype(np.float32)

    blob = np.zeros((128, BLOB_W), np.float32)
    blob[0:65, _WVB0:_WVB0 + 66] = wvb
    blob[0:65, _W2K0:_W2K0 + 24] = w2(wk)
    blob[0:65, _W2Q0:_W2Q0 + 24] = w2(wq)
    blob[0:24, _ACOL] = afull
    blob[:, _IDEN:_IDEN + 128] = np.eye(128, dtype=np.float32)

    ones = np.ones((N, 1), np.float32)
    in_maps = []
    for c in range(NCORES):
        b, qoff = c // 2, (c % 2) * NQ
        xr = np.concatenate([x[b, qoff:], x[b, :qoff]], axis=0) if qoff else x[b]
        xth1 = np.concatenate([xr.T, ones.T], 0).astype(np.float32)
        xaf = np.concatenate([xr, ones], 1)
        xa = xaf.astype(ml_dtypes.bfloat16)
        xl = (xr[0:NQ] - xa[0:NQ, 0:DK].astype(np.float32)).astype(
            ml_dtypes.bfloat16)
        in_maps.append({"xth1": np.ascontiguousarray(xth1),
                        "xa": np.ascontiguousarray(xa),
                        "xl": np.ascontiguousarray(xl),
                        "blob": blob})
    return in_maps


def kernel(x, Wv, bv, wq, wk, gamma, beta, _trace=False, _trace_cores=None):
    nc = _get_nc()
    in_maps = make_in_maps(x, Wv, bv, wq, wk, gamma, beta)
    res = run_bass_kernel_spmd(nc, in_maps, core_ids=list(range(NCORES)),
                               trace=_trace, trace_cores=_trace_cores)
    out = np.empty((B, N, DK), np.float32)
    for c in range(NCORES):
        b, qoff = c // 2, (c % 2) * NQ
        out[b, qoff:qoff + NQ] = res.results[c]["out"]
    # gamma/beta are ones/zeros in this problem's setup; apply on host if not.
    g = np.asarray(gamma, np.float32)
    bt = np.asarray(beta, np.float32)
    if not (np.all(g == 1.0) and np.all(bt == 0.0)):
        out = out * g + bt
    kernel._last_results = res
    return out


# revision 11
# speedup vs baseline: 1.7537x; 1.3294x over previous
"""Trainium2 Bass kernel for nn_AttentionHead_Hybrid2 (B=4, N=4096, DK=64).

reference:
    V = x @ Wv.T + bv              (B,N,DK)
    Q = x @ wq ; K = x @ wk        (B,N)
    A = exp(-(Q_i - K_j)^2)        (B,N,N)
    P = softmax(A / 8, axis=-1)
    out = LN(P @ V + x)

Sharding: 8 cores = (batch b = c//2) x (query half c%2). Each core gets the
full key set for its batch (rolled so its 2048 queries are rows 0:2048) and
produces its 2048x64 output slice.

Algorithm (Fourier separation): the score E(q,k) = exp(exp(-(q-k)^2)/8)
depends only on t = q - k, so it has a rapidly-converging cosine expansion
E(t) = sum_k a_k cos(w_k t) (periodized, L=13, 24 cos/sin features gives
~3e-5 abs accuracy). cos(w(Q-K)) = cosQcosK + sinQsinK makes attention
separable with rank 24:
    num (2048, 66) = PhiQ (2048,24) @ [ a*(Wv-transformed PhiK-moments) ]
where PhiK/PhiQ are sin/cos feature maps of the key/query scalar
projections; col 64 = softmax denominator, col 65 = numerator row-sum
(for LN stats). No (N,N) scores, no binning, no big exp fields.

Phases are computed in turns r = u/2pi directly from x: per 128-token tile,
u_tile = xth_tile.T @ W2 with W2 = w (x) k/L + phase-row (bf16 hi/lo split;
residual phase errors are incoherent across keys and wash out). The ACT sin
table is only valid on [-pi,pi], so one DVE magic-number round pass forms
w = r - round(r) in [-0.5,0.5] and the ACT evaluates sin(2pi*w).

Query features are computed tile-major like keys (sharing the stationary
xth tile), then PE-transposed (bf16) to features-on-partitions so each
numerator matmul lands tokens-on-partitions - the LN tail needs no
transposes at all.

LayerNorm is scale-invariant, so no division by the softmax denominator:
z = num + den*x, out = (z - mean) * rsqrt(E[z^2] - mean^2); gamma/beta are
ones/zeros per the problem spec (host applies them if they ever are not).
Ln/Exp are batched once so the ACT table (trig <-> ln/exp set) switches
exactly once mid-kernel.
"""

import math
import sys

for _p in ("/opt/trn_rl_repo", "/root/.axon_site/_ro/trn_rl_repo"):
    if _p not in sys.path:
        sys.path.insert(0, _p)

import numpy as np

import concourse.bass as bass
import concourse.mybir as mybir
import concourse.tile as tile
import bass_rust
from concourse.bass_utils import run_bass_kernel_spmd

F32 = mybir.dt.float32
BF16 = mybir.dt.bfloat16
AF = mybir.ActivationFunctionType
OP = mybir.AluOpType

B, N, DK = 4, 4096, 64
NQ = 2048          # queries per core
NCORES = 8
JT = N // 128      # 32 key tiles
IT = NQ // 128     # 16 query tiles
LPER = 13.0        # Fourier period in t = q - k
NF = 24            # features: cos k=0..12, sin k=1..11
MAGIC = float(np.float32(1.5 * 2 ** 23))   # fp32 round-to-nearest trick
GRP = 16           # phase tiles per round/sin group

# const blob column layout (fp32)
_WVB0 = 0                 # (65, 66)
_ACOL = _WVB0 + 66        # (24, 1)  feature coefficients
BLOB_W = _ACOL + 1


def split_multiwaits(nc):
    """Walrus in this env accepts one sem-wait per instruction; Tile emits
    several. Split extras onto preceding same-engine NoOps."""
    ctr = 0
    for f in nc.m.functions:
        for bb in f.blocks:
            out, changed = [], False
            for ins in bb.instructions:
                si = ins.sync_info
                if si is not None and si.on_wait and len(si.on_wait) > 1:
                    waits = list(si.on_wait)
                    for w in waits[:-1]:
                        ctr += 1
                        out.append(mybir.InstNoOp(
                            name=f"I-wsplit-{ctr}", engine=ins.engine,
                            debug=ins.debug, ins=[], outs=[],
                            sync_info=bass_rust.SyncInfo(on_wait=[w], on_update=[])))
                    ins.sync_info = bass_rust.SyncInfo(
                        on_wait=[waits[-1]], on_update=list(si.on_update or []))
                    changed = True
                out.append(ins)
            if changed:
                bb.instructions = out
    return ctr


def build_nc(split=True):
    nc = bass.Bass("TRN2", target_bir_lowering=False, debug=False)

    blob_d = nc.dram_tensor("blob", [128, BLOB_W], F32, kind="ExternalInput").ap()
    wf_d = nc.dram_tensor("wf", [65, 4 * NF], BF16, kind="ExternalInput").ap()
    idb_d = nc.dram_tensor("idb", [128, 128], BF16, kind="ExternalInput").ap()
    xth_d = nc.dram_tensor("xth", [DK + 1, N], BF16, kind="ExternalInput").ap()
    xa_d = nc.dram_tensor("xa", [128, JT * 65], BF16, kind="ExternalInput").ap()
    xl_d = nc.dram_tensor("xl", [128, IT * DK], BF16, kind="ExternalInput").ap()
    out_d = nc.dram_tensor("out", [NQ, DK], F32, kind="ExternalOutput").ap()

    with tile.TileContext(nc) as tc:
        cpool = tc.alloc_tile_pool(name="consts", bufs=1)
        big = tc.alloc_tile_pool(name="big", bufs=1)

        blob = cpool.tile([128, BLOB_W], F32)
        nc.sync.dma_start(blob[:], blob_d[:])
        wf = cpool.tile([65, 4 * NF], BF16)
        nc.sync.dma_start(wf[:], wf_d[:])
        idb = cpool.tile([128, 128], BF16)
        nc.sync.dma_start(idb[:], idb_d[:])
        wvb = blob[0:65, _WVB0:_WVB0 + 66]
        acol = blob[0:24, _ACOL:_ACOL + 1]
        w2kh = wf[:, 0 * NF:1 * NF]
        w2kl = wf[:, 1 * NF:2 * NF]
        w2qh = wf[:, 2 * NF:3 * NF]
        w2ql = wf[:, 3 * NF:4 * NF]

        xth = big.tile([DK + 1, N], BF16)
        for h in (4, 5, 6, 7, 0, 1, 2, 3):      # tiles 16..31 are consumed first
            nc.sync.dma_start(xth[:, h * 512:(h + 1) * 512],
                              xth_d[:, h * 512:(h + 1) * 512])

        # host-pretiled natural layouts: [p, tile, col]
        xa_all = big.tile([128, JT * 65], BF16)
        xa_v = xa_all.rearrange("p (t c) -> p t c", c=65)
        xl_all = big.tile([128, IT * DK], BF16)
        xl_v = xl_all.rearrange("p (t c) -> p t c", c=DK)
        for h in range(4):
            nc.gpsimd.dma_start(xa_all[:, h * 520:(h + 1) * 520],
                                xa_d[:, h * 520:(h + 1) * 520])
        nc.gpsimd.dma_start(xl_all[:], xl_d[:])

        # ---- phase features ----
        # groups: 0 = keys 16..31, 1 = keys 0..15, 2 = queries (= tiles 0..15)
        phk = big.tile([128, JT * NF], BF16)      # key features, tile-major
        phk_v = phk.rearrange("p (t f) -> p t f", f=NF)
        phqt = big.tile([128, IT * NF], BF16)     # query features, tile-major
        phqt_v = phqt.rearrange("p (t f) -> p t f", f=NF)
        phq = big.tile([24, NQ], BF16)            # query features, rotated
        w_sb = big.tile([128, 3 * GRP * NF], F32)  # reduced phases w = r - rt
        w_v = w_sb.rearrange("p (g c) -> p g c", c=GRP * NF)

        with (tc.tile_pool(name="u_ps", bufs=3, space="PSUM") as ups,
              tc.tile_pool(name="pq_ps", bufs=4, space="PSUM") as pqps,
              tc.tile_pool(name="rt_sb", bufs=3) as rtp):
            u1 = ups.tile([128, GRP * NF], F32, tag="u")
            u1_t = u1.rearrange("p (t f) -> p t f", f=NF)
            for i in range(GRP):                  # keys 16..31 first
                sl = xth[:, (GRP + i) * 128:(GRP + i + 1) * 128]
                nc.tensor.matmul(u1_t[:, i, :], sl, w2kh, start=True, stop=False)
                nc.tensor.matmul(u1_t[:, i, :], sl, w2kl, start=False, stop=True)
            u0 = ups.tile([128, GRP * NF], F32, tag="u")
            u0_t = u0.rearrange("p (t f) -> p t f", f=NF)
            u2 = ups.tile([128, GRP * NF], F32, tag="u")
            u2_t = u2.rearrange("p (t f) -> p t f", f=NF)
            for i in range(GRP):                  # keys 0..15 + queries
                sl = xth[:, i * 128:(i + 1) * 128]
                nc.tensor.matmul(u0_t[:, i, :], sl, w2kh, start=True, stop=False)
                nc.tensor.matmul(u0_t[:, i, :], sl, w2kl, start=False, stop=True)
                nc.tensor.matmul(u2_t[:, i, :], sl, w2qh, start=True, stop=False)
                nc.tensor.matmul(u2_t[:, i, :], sl, w2ql, start=False, stop=True)

            for g, u in ((0, u1), (1, u0), (2, u2)):
                rt = rtp.tile([128, GRP * NF], F32, tag="rt")
                nc.vector.tensor_scalar(rt[:], u[:], MAGIC, MAGIC, OP.add,
                                        OP.subtract)
                nc.vector.tensor_tensor(w_v[:, g, :], u[:], rt[:], OP.subtract)
                if g == 0:
                    dst = phk[:, GRP * NF:2 * GRP * NF]
                elif g == 1:
                    dst = phk[:, 0:GRP * NF]
                else:
                    dst = phqt[:]
                nc.scalar.activation(dst, w_v[:, g, :], AF.Sin,
                                     scale=2 * math.pi)

            # transpose query features to (24, NQ), bf16
            for q in range(4):
                pt = pqps.tile([24, 512], BF16, tag="pt")
                for i in range(4):
                    nc.tensor.transpose(pt[:, i * 128:(i + 1) * 128],
                                        phqt_v[:, q * 4 + i, :], idb)
                nc.vector.tensor_copy(phq[:, q * 512:(q + 1) * 512], pt[:])

        # ---- key moments F -> coefficient-folded Fw (hi/lo bf16) ----
        fwh = big.tile([24, 66], BF16)
        fwl = big.tile([24, 66], BF16)
        with tc.tile_pool(name="f_ps", bufs=2, space="PSUM") as fps:
            f_ps = fps.tile([65, NF], F32, tag="f")
            f_order = list(range(GRP, JT)) + list(range(GRP))
            for n_, jt in enumerate(f_order):
                nc.tensor.matmul(f_ps[:], xa_v[:, jt, :], phk_v[:, jt, :],
                                 start=(n_ == 0), stop=(n_ == JT - 1))
            f_sb = big.tile([65, NF], F32)
            nc.vector.tensor_copy(f_sb[:], f_ps[:])
            fw_ps = fps.tile([24, 66], F32, tag="fw")
            nc.tensor.matmul(fw_ps[:], f_sb[:], wvb, start=True, stop=True)
            fwm = big.tile([24, 66], F32)
            nc.vector.tensor_tensor(fwm[:], fw_ps[:],
                                    acol.broadcast_to([24, 66]), OP.mult)
            nc.vector.tensor_copy(fwh[:], fwm[:])
            nc.vector.tensor_tensor(fwl[:], fwm[:], fwh[:], OP.subtract)

        # residual x (fp32) and per-token row sums, off the critical path
        xq = big.tile([128, IT * DK], F32)
        xq_v = xq.rearrange("p (t d) -> p t d", d=DK)
        xsum = big.tile([128, IT], F32)
        nc.gpsimd.tensor_tensor(xq_v[:], xa_v[:, 0:IT, 0:DK], xl_v[:], OP.add)
        nc.vector.reduce_sum(xsum[:], xq_v[:], axis=mybir.AxisListType.X)

        # ---- numerator (tokens on partitions) + fused LN tail ----
        z_sb = big.tile([128, IT * DK], F32)
        z_v = z_sb.rearrange("p (t d) -> p t d", d=DK)
        o_sb = big.tile([128, IT * DK], F32)
        o_v = o_sb.rearrange("p (t d) -> p t d", d=DK)
        sq = big.tile([128, 8 * DK], F32)
        sq_v = sq.rearrange("p (t d) -> p t d", d=DK)
        st = big.tile([128, 8 * IT], F32)   # stats: s2, mu, var, rstd, -mu*rstd
        s2 = st[:, 0 * IT:1 * IT]
        mu_c = st[:, 2 * IT:3 * IT]
        var_c = st[:, 3 * IT:4 * IT]
        rstd_c = st[:, 4 * IT:5 * IT]
        nmu_c = st[:, 5 * IT:6 * IT]
        t1 = big.tile([128, 8 * DK], F32)
        t1_v = t1.rearrange("p (t d) -> p t d", d=DK)

        with tc.tile_pool(name="num_ps", bufs=1, space="PSUM") as nps:
            nf = nps.tile([128, IT * 128], F32)
            nf_v = nf.rearrange("p (t c) -> p t c", c=128)
            for it in range(IT):
                lhs = phq[:, it * 128:(it + 1) * 128]
                nc.tensor.matmul(nf_v[:, it, 0:66], lhs, fwh[:],
                                 start=True, stop=False)
                nc.tensor.matmul(nf_v[:, it, 0:66], lhs, fwl[:],
                                 start=False, stop=True)
            for h in range(4):
                ts_, te_ = h * 4, (h + 1) * 4
                sl = slice((h % 2) * 4, (h % 2) * 4 + 4)
                nv = nf_v[:, ts_:te_, :]
                mu3 = mu_c[:, ts_:te_].unsqueeze(-1)
                # z = num + den * x
                nc.vector.tensor_tensor(
                    t1_v[:, sl, :], xq_v[:, ts_:te_, :],
                    nv[:, :, 64:65].broadcast_to([128, 4, DK]), OP.mult)
                nc.vector.tensor_tensor(z_v[:, ts_:te_, :], t1_v[:, sl, :],
                                        nv[:, :, 0:64], OP.add)
                # sum z^2 (Square is in the trig table set: no table switch)
                nc.scalar.activation(sq_v[:, sl, :].rearrange("p t d -> p (t d)"),
                                     z_v[:, ts_:te_, :].rearrange("p t d -> p (t d)"),
                                     AF.Square, scale=1.0)
                nc.vector.reduce_sum(s2[:, ts_:te_], sq_v[:, sl, :],
                                     axis=mybir.AxisListType.X)
                # mu*64 = numsum + den*xsum
                nc.vector.tensor_tensor(mu3, nv[:, :, 64:65],
                                        xsum[:, ts_:te_].unsqueeze(-1), OP.mult)
                nc.vector.tensor_tensor(mu3, mu3, nv[:, :, 65:66], OP.add)
            # batched stats: mu, var = s2/64 - mu^2, rstd = exp(-.5 ln var)
            nc.vector.tensor_scalar_mul(mu_c[:], mu_c[:], 1.0 / DK)
            nc.vector.tensor_tensor(var_c[:], mu_c[:], mu_c[:], OP.mult)
            nc.vector.scalar_tensor_tensor(var_c[:], s2[:], 1.0 / DK, var_c[:],
                                           OP.mult, OP.subtract)
            nc.scalar.activation(rstd_c[:], var_c[:], AF.Ln, scale=1.0)
            nc.scalar.activation(rstd_c[:], rstd_c[:], AF.Exp, scale=-0.5)
            nc.vector.tensor_tensor(nmu_c[:], mu_c[:], rstd_c[:], OP.mult)
            nc.vector.tensor_scalar_mul(nmu_c[:], nmu_c[:], -1.0)
            for h in range(4):
                ts_, te_ = h * 4, (h + 1) * 4
                # out = z*rstd - mu*rstd
                nc.vector.tensor_tensor(
                    o_v[:, ts_:te_, :], z_v[:, ts_:te_, :],
                    rstd_c[:, ts_:te_].unsqueeze(-1).broadcast_to([128, 4, DK]),
                    OP.mult)
                nc.vector.tensor_tensor(
                    o_v[:, ts_:te_, :], o_v[:, ts_:te_, :],
                    nmu_c[:, ts_:te_].unsqueeze(-1).broadcast_to([128, 4, DK]),
                    OP.add)
                nc.sync.dma_start(
                    out_d[h * 512:(h + 1) * 512, :].rearrange(
                        "(t p) d -> p t d", p=128), o_v[:, ts_:te_, :])

        big.release()
        cpool.release()

    if split:
        split_multiwaits(nc)
    return nc


_NC_CACHE = None


def _get_nc():
    global _NC_CACHE
    if _NC_CACHE is None:
        _NC_CACHE = build_nc()
    return _NC_CACHE


def _fourier_coeffs():
    m = 16384
    t = LPER * np.arange(m) / m
    tw = np.minimum(t, LPER - t)
    g = np.exp(np.exp(-tw ** 2) / 8.0) - 1.0
    c = np.fft.rfft(g) / m
    a_cos = np.concatenate([[1.0 + np.real(c[0])], 2 * np.real(c[1:13])])
    a_sin = 2 * np.real(c[1:12])
    return np.concatenate([a_cos, a_sin]).astype(np.float32)


def make_in_maps(x, Wv, bv, wq, wk, gamma, beta):
    import ml_dtypes
    bf = ml_dtypes.bfloat16
    x = np.asarray(x, np.float32)
    kfeat = np.concatenate([np.arange(13), np.arange(1, 12)]).astype(np.float64)
    phip = np.concatenate([0.25 * np.ones(13), np.zeros(11)])

    wvb = np.zeros((65, 66), np.float32)
    wvb[:64, :64] = np.asarray(Wv, np.float32).T
    wvb[64, :64] = np.asarray(bv, np.float32)
    wvb[64, 64] = 1.0
    wvb[:, 65] = wvb[:, :64].sum(1)

    blob = np.zeros((128, BLOB_W), np.float32)
    blob[0:65, _WVB0:_WVB0 + 66] = wvb
    blob[0:24, _ACOL] = _fourier_coeffs()

    def w2pair(w):
        full = np.concatenate(
            [np.outer(np.asarray(w, np.float64), kfeat / LPER),
             phip[None, :]], 0).astype(np.float32)
        hi = full.astype(bf)
        lo = (full - hi.astype(np.float32)).astype(bf)
        return hi, lo

    wkh, wkl = w2pair(wk)
    wqh, wql = w2pair(wq)
    wf = np.concatenate([wkh, wkl, wqh, wql], 1).astype(bf)
    idb = np.eye(128, dtype=bf)

    ones = np.ones((N, 1), np.float32)
    in_maps = []
    for c in range(NCORES):
        b, qoff = c // 2, (c % 2) * NQ
        xr = np.concatenate([x[b, qoff:], x[b, :qoff]], axis=0) if qoff else x[b]
        xth = np.concatenate([xr.T, ones.T], 0).astype(bf)
        xaf = np.concatenate([xr, ones], 1)
        xa = xaf.astype(bf)                                  # (N, 65)
        xl = (xr[0:NQ] - xa[0:NQ, 0:DK].astype(np.float32)).astype(bf)
        # pre-tile to [p, tile, col] so device DMAs are contiguous
        xa_t = np.ascontiguousarray(
            xa.reshape(JT, 128, 65).transpose(1, 0, 2).reshape(128, JT * 65))
        xl_t = np.ascontiguousarray(
            xl.reshape(IT, 128, DK).transpose(1, 0, 2).reshape(128, IT * DK))
        in_maps.append({"xth": np.ascontiguousarray(xth),
                        "xa": xa_t, "xl": xl_t,
                        "wf": wf, "idb": idb, "blob": blob})
    return in_maps


def kernel(x, Wv, bv, wq, wk, gamma, beta, _trace=False, _trace_cores=None):
    nc = _get_nc()
    in_maps = make_in_maps(x, Wv, bv, wq, wk, gamma, beta)
    res = run_bass_kernel_spmd(nc, in_maps, core_ids=list(range(NCORES)),
                               trace=_trace, trace_cores=_trace_cores)
    out = np.empty((B, N, DK), np.float32)
    for c in range(NCORES):
        b, qoff = c // 2, (c % 2) * NQ
        out[b, qoff:qoff + NQ] = res.results[c]["out"]
    # gamma/beta are ones/zeros in this problem's setup; apply on host if not.
    g = np.asarray(gamma, np.float32)
    bt = np.asarray(beta, np.float32)
    if not (np.all(g == 1.0) and np.all(bt == 0.0)):
        out = out * g + bt
    kernel._last_results = res
    return out


# revision 20
# speedup vs baseline: 2.1034x; 1.1994x over previous
"""Trainium2 Bass kernel for nn_AttentionHead_Hybrid2 (B=4, N=4096, DK=64).

reference:
    V = x @ Wv.T + bv              (B,N,DK)
    Q = x @ wq ; K = x @ wk        (B,N)
    A = exp(-(Q_i - K_j)^2)        (B,N,N)
    P = softmax(A / 8, axis=-1)
    out = LN(P @ V + x)

Sharding: 8 cores = (batch b = c//2) x (query half c%2). Each core gets the
full key set for its batch (rolled so its 2048 queries are rows 0:2048) and
produces its 2048x64 output slice.

Algorithm (Fourier separation): the score E(q,k) = exp(exp(-(q-k)^2)/8)
depends only on t = q - k, so it has a rapidly-converging cosine expansion
E(t) = sum_k a_k cos(w_k t) (periodized, L=13, 24 cos/sin features gives
~3e-5 abs accuracy). cos(w(Q-K)) = cosQcosK + sinQsinK makes attention
separable with rank 24:
    num (2048, 66) = PhiQ (2048,24) @ [ a*(Wv-transformed PhiK-moments) ]
where PhiK/PhiQ are sin/cos feature maps of the key/query scalar
projections; col 64 = softmax denominator, col 65 = numerator row-sum
(for LN stats). No (N,N) scores, no binning, no big exp fields.

Phases are computed in turns r = u/2pi directly from x: per 128-token tile,
u_tile = xth_tile.T @ W2 with W2 = w (x) k/L + phase-row (bf16 hi/lo split;
residual phase errors are incoherent across keys and wash out). The ACT sin
table is only valid on [-pi,pi], so one DVE magic-number round pass forms
w = r - round(r) in [-0.5,0.5] and the ACT evaluates sin(2pi*w).

Query features are computed tile-major like keys (sharing the stationary
xth tile), then PE-transposed (bf16) to features-on-partitions so each
numerator matmul lands tokens-on-partitions - the LN tail needs no
transposes at all.

LayerNorm is scale-invariant, so no division by the softmax denominator:
z = num + den*x, out = (z - mean) * rsqrt(E[z^2] - mean^2); gamma/beta are
ones/zeros per the problem spec (host applies them if they ever are not).
Ln/Exp are batched once so the ACT table (trig <-> ln/exp set) switches
exactly once mid-kernel.
"""

import math
import sys

for _p in ("/opt/trn_rl_repo", "/root/.axon_site/_ro/trn_rl_repo"):
    if _p not in sys.path:
        sys.path.insert(0, _p)

import numpy as np

import concourse.bass as bass
import concourse.mybir as mybir
import concourse.tile as tile
import bass_rust
from concourse.bass_utils import run_bass_kernel_spmd

F32 = mybir.dt.float32
BF16 = mybir.dt.bfloat16
AF = mybir.ActivationFunctionType
OP = mybir.AluOpType

B, N, DK = 4, 4096, 64
NQ = 2048          # queries per core
NCORES = 8
JT = N // 128      # 32 key tiles
IT = NQ // 128     # 16 query tiles
LPER = 13.0        # Fourier period in t = q - k
NF = 24            # features: cos k=0..12, sin k=1..11
MAGIC = float(np.float32(1.5 * 2 ** 23))   # fp32 round-to-nearest trick
GRP = 16           # phase tiles per round/sin group

# const blob column layout (fp32)
_WVB0 = 0                 # (65, 66)
_ACOL = _WVB0 + 66        # (24, 1)  feature coefficients
BLOB_W = _ACOL + 1


def split_multiwaits(nc):
    """Walrus in this env accepts one sem-wait per instruction; Tile emits
    several. Split extras onto preceding same-engine NoOps."""
    ctr = 0
    for f in nc.m.functions:
        for bb in f.blocks:
            out, changed = [], False
            for ins in bb.instructions:
                si = ins.sync_info
                if si is not None and si.on_wait and len(si.on_wait) > 1:
                    waits = list(si.on_wait)
                    for w in waits[:-1]:
                        ctr += 1
                        out.append(mybir.InstNoOp(
                            name=f"I-wsplit-{ctr}", engine=ins.engine,
                            debug=ins.debug, ins=[], outs=[],
                            sync_info=bass_rust.SyncInfo(on_wait=[w], on_update=[])))
                    ins.sync_info = bass_rust.SyncInfo(
                        on_wait=[waits[-1]], on_update=list(si.on_update or []))
                    changed = True
                out.append(ins)
            if changed:
                bb.instructions = out
    return ctr


def build_nc(split=True):
    nc = bass.Bass("TRN2", target_bir_lowering=False, debug=False)

    blob_d = nc.dram_tensor("blob", [128, BLOB_W], F32, kind="ExternalInput").ap()
    wf_d = nc.dram_tensor("wf", [65, 4 * NF + 1], BF16, kind="ExternalInput").ap()
    idb_d = nc.dram_tensor("idb", [128, 128], BF16, kind="ExternalInput").ap()
    xth_d = nc.dram_tensor("xth", [DK + 1, N], BF16, kind="ExternalInput").ap()
    xa_d = nc.dram_tensor("xa", [128, JT * 65], BF16, kind="ExternalInput").ap()
    xl_d = nc.dram_tensor("xl", [128, IT * DK], BF16, kind="ExternalInput").ap()
    out_d = nc.dram_tensor("out", [128, IT * DK], F32, kind="ExternalOutput").ap()

    with tile.TileContext(nc) as tc:
        cpool = tc.alloc_tile_pool(name="consts", bufs=1)
        big = tc.alloc_tile_pool(name="big", bufs=1)

        blob = cpool.tile([128, BLOB_W], F32)
        nc.sync.dma_start(blob[:], blob_d[:])
        wf = cpool.tile([65, 4 * NF + 1], BF16)
        nc.sync.dma_start(wf[:], wf_d[:])
        wvb = blob[0:65, _WVB0:_WVB0 + 66]
        acol = blob[0:24, _ACOL:_ACOL + 1]
        w2kh = wf[:, 0 * NF:1 * NF]
        w2kl = wf[:, 1 * NF:2 * NF]
        w2qh = wf[:, 2 * NF:3 * NF]
        w2ql = wf[:, 3 * NF:4 * NF]
        csel = wf[:, 4 * NF:4 * NF + 1]          # [1]*64 + [0]: row-sum select

        xth = big.tile([DK + 1, N], BF16)
        for h in (4, 5, 6, 7, 0, 1, 2, 3):      # tiles 16..31 are consumed first
            nc.sync.dma_start(xth[:, h * 512:(h + 1) * 512],
                              xth_d[:, h * 512:(h + 1) * 512])

        idb = cpool.tile([128, 128], BF16)
        nc.sync.dma_start(idb[:], idb_d[:])

        # host-pretiled natural layouts: [p, tile, col]
        xa_all = big.tile([128, JT * 65], BF16)
        xa_v = xa_all.rearrange("p (t c) -> p t c", c=65)
        xl_all = big.tile([128, IT * DK], BF16)
        xl_v = xl_all.rearrange("p (t c) -> p t c", c=DK)
        for h in range(4):
            nc.sync.dma_start(xa_all[:, h * 520:(h + 1) * 520],
                              xa_d[:, h * 520:(h + 1) * 520])
        nc.sync.dma_start(xl_all[:], xl_d[:])

        # ---- phase features ----
        # groups: 0 = keys 16..31, 1 = keys 0..15, 2 = queries (= tiles 0..15)
        phk = big.tile([128, JT * NF], BF16)      # key features, tile-major
        phk_v = phk.rearrange("p (t f) -> p t f", f=NF)
        phqt = big.tile([128, IT * NF], BF16)     # query features, tile-major
        phqt_v = phqt.rearrange("p (t f) -> p t f", f=NF)
        phq = big.tile([24, NQ], BF16)            # query features, rotated
        w_sb = big.tile([128, 3 * GRP * NF], F32)  # reduced phases w = r - rt
        w_v = w_sb.rearrange("p (g c) -> p g c", c=GRP * NF)

        with (tc.tile_pool(name="u_ps", bufs=3, space="PSUM") as ups,
              tc.tile_pool(name="pq_ps", bufs=4, space="PSUM") as pqps,
              tc.tile_pool(name="facc_ps", bufs=1, space="PSUM") as faccp,
              tc.tile_pool(name="rt_sb", bufs=3) as rtp):
            u1 = ups.tile([128, GRP * NF], F32, tag="u")
            u1_t = u1.rearrange("p (t f) -> p t f", f=NF)
            for i in range(GRP):                  # keys 16..31 first
                sl = xth[:, (GRP + i) * 128:(GRP + i + 1) * 128]
                nc.tensor.matmul(u1_t[:, i, :], sl, w2kh, start=True, stop=False)
                nc.tensor.matmul(u1_t[:, i, :], sl, w2kl, start=False, stop=True)
            u0 = ups.tile([128, GRP * NF], F32, tag="u")
            u0_t = u0.rearrange("p (t f) -> p t f", f=NF)
            u2 = ups.tile([128, GRP * NF], F32, tag="u")
            u2_t = u2.rearrange("p (t f) -> p t f", f=NF)
            for i in range(GRP):                  # keys 0..15 + queries
                sl = xth[:, i * 128:(i + 1) * 128]
                nc.tensor.matmul(u0_t[:, i, :], sl, w2kh, start=True, stop=False)
                nc.tensor.matmul(u0_t[:, i, :], sl, w2kl, start=False, stop=True)
                nc.tensor.matmul(u2_t[:, i, :], sl, w2qh, start=True, stop=False)
                nc.tensor.matmul(u2_t[:, i, :], sl, w2ql, start=False, stop=True)

            fwh = big.tile([24, 66], BF16)
            fwl = big.tile([24, 66], BF16)
            f_sb = big.tile([65, NF], F32)
            f_ps = faccp.tile([65, NF], F32, tag="f")
            dummy = big.tile([1, 1], F32)

            for g, u in ((0, u1), (1, u0), (2, u2)):
                rt = rtp.tile([128, GRP * NF], F32, tag="rt")
                nc.vector.tensor_scalar(rt[:], u[:], MAGIC, MAGIC, OP.add,
                                        OP.subtract)
                nc.vector.tensor_tensor(w_v[:, g, :], u[:], rt[:], OP.subtract)
                if g == 0:
                    dst = phk[:, GRP * NF:2 * GRP * NF]
                elif g == 1:
                    dst = phk[:, 0:GRP * NF]
                else:
                    dst = phqt[:]
                nc.scalar.activation(dst, w_v[:, g, :], AF.Sin,
                                     scale=2 * math.pi)
                if g == 0:
                    # key moments for tiles 16..31 while queries still cook
                    for jt in range(GRP, JT):
                        nc.tensor.matmul(f_ps[:], xa_v[:, jt, :],
                                         phk_v[:, jt, :],
                                         start=(jt == GRP), stop=False)

            # trigger the trig -> ln/exp ACT table switch off the critical path
            nc.scalar.activation(dummy[:], w_sb[0:1, 0:1], AF.Ln, scale=1.0)

            # transpose query features to (24, NQ), bf16
            for q in range(4):
                pt = pqps.tile([24, 512], BF16, tag="pt")
                for i in range(4):
                    nc.tensor.transpose(pt[:, i * 128:(i + 1) * 128],
                                        phqt_v[:, q * 4 + i, :], idb)
                nc.vector.tensor_copy(phq[:, q * 512:(q + 1) * 512], pt[:])

            for jt in range(GRP):
                nc.tensor.matmul(f_ps[:], xa_v[:, jt, :], phk_v[:, jt, :],
                                 start=False, stop=(jt == GRP - 1))
            nc.vector.tensor_copy(f_sb[:], f_ps[:])

        # ---- Fw = (F.T @ WVB) * a, split hi/lo bf16 ----
        with tc.tile_pool(name="fw_ps", bufs=1, space="PSUM") as fps:
            fw_ps = fps.tile([24, 66], F32, tag="fw")
            nc.tensor.matmul(fw_ps[:], f_sb[:], wvb, start=True, stop=True)
            fwm = big.tile([24, 66], F32)
            nc.vector.tensor_tensor(fwm[:], fw_ps[:],
                                    acol.broadcast_to([24, 66]), OP.mult)
            nc.vector.tensor_copy(fwh[:], fwm[:])
            nc.vector.tensor_tensor(fwl[:], fwm[:], fwh[:], OP.subtract)

        # residual x (fp32), off the critical path on gpsimd
        xq = big.tile([128, IT * DK], F32)
        xq_v = xq.rearrange("p (t d) -> p t d", d=DK)
        nc.gpsimd.tensor_tensor(xq_v[:], xa_v[:, 0:IT, 0:DK], xl_v[:], OP.add)

        # ---- numerator (tokens on partitions) + fused LN tail ----
        z_sb = big.tile([128, IT * DK], F32)
        z_v = z_sb.rearrange("p (t d) -> p t d", d=DK)
        o_sb = big.tile([128, IT * DK], F32)
        o_v = o_sb.rearrange("p (t d) -> p t d", d=DK)
        sq = big.tile([128, 8 * DK], F32)
        sq_v = sq.rearrange("p (t d) -> p t d", d=DK)
        st = big.tile([128, 8 * IT], F32)   # stats: s2, mu, var, rstd, -mu*rstd
        s2 = st[:, 0 * IT:1 * IT]
        mu_c = st[:, 2 * IT:3 * IT]
        var_c = st[:, 3 * IT:4 * IT]
        rstd_c = st[:, 4 * IT:5 * IT]
        nmu_c = st[:, 5 * IT:6 * IT]
        t1 = big.tile([128, 8 * DK], F32)
        t1_v = t1.rearrange("p (t d) -> p t d", d=DK)

        with tc.tile_pool(name="num_ps", bufs=4, space="PSUM") as nps:
            nvs = []
            for h in range(4):
                nf = nps.tile([128, 4 * 128], F32, tag="nf")
                nf_v = nf.rearrange("p (t c) -> p t c", c=128)
                nvs.append(nf_v)
                for i in range(4):
                    it = h * 4 + i
                    lhs = phq[:, it * 128:(it + 1) * 128]
                    nc.tensor.matmul(nf_v[:, i, 0:66], lhs, fwh[:],
                                     start=True, stop=False)
                    nc.tensor.matmul(nf_v[:, i, 0:66], lhs, fwl[:],
                                     start=False, stop=True)
                    # per-token row sum of x rides in col 66 (for LN mean)
                    nc.tensor.matmul(nf_v[:, i, 66:67],
                                     xth[:, it * 128:(it + 1) * 128], csel,
                                     start=True, stop=True)
            for h in range(4):
                ts_, te_ = h * 4, (h + 1) * 4
                sl = slice((h % 2) * 4, (h % 2) * 4 + 4)
                nv = nvs[h]
                mu3 = mu_c[:, ts_:te_].unsqueeze(-1)
                # z = num + den * x
                nc.vector.tensor_tensor(
                    t1_v[:, sl, :], xq_v[:, ts_:te_, :],
                    nv[:, :, 64:65].broadcast_to([128, 4, DK]), OP.mult)
                nc.vector.tensor_tensor(z_v[:, ts_:te_, :], t1_v[:, sl, :],
                                        nv[:, :, 0:64], OP.add)
                # sum z^2 (Square is in every ACT table set: no switch)
                nc.scalar.activation(sq_v[:, sl, :].rearrange("p t d -> p (t d)"),
                                     z_v[:, ts_:te_, :].rearrange("p t d -> p (t d)"),
                                     AF.Square, scale=1.0)
                nc.vector.reduce_sum(s2[:, ts_:te_], sq_v[:, sl, :],
                                     axis=mybir.AxisListType.X)
                # mu*64 = numsum + den*xsum  (den staged to SBUF: DVE reads
                # at most one PSUM operand per instruction)
                den3 = st[:, 6 * IT + ts_:6 * IT + te_].unsqueeze(-1)
                nc.vector.tensor_copy(den3, nv[:, :, 64:65])
                nc.vector.tensor_tensor(mu3, den3, nv[:, :, 66:67], OP.mult)
                nc.vector.tensor_tensor(mu3, mu3, nv[:, :, 65:66], OP.add)
            # batched stats: mu, var = s2/64 - mu^2, rstd = exp(-.5 ln var)
            nc.vector.tensor_scalar_mul(mu_c[:], mu_c[:], 1.0 / DK)
            nc.vector.tensor_tensor(var_c[:], mu_c[:], mu_c[:], OP.mult)
            nc.vector.scalar_tensor_tensor(var_c[:], s2[:], 1.0 / DK, var_c[:],
                                           OP.mult, OP.subtract)
            nc.scalar.activation(rstd_c[:], var_c[:], AF.Ln, scale=1.0)
            nc.scalar.activation(rstd_c[:], rstd_c[:], AF.Exp, scale=-0.5)
            nc.vector.tensor_tensor(nmu_c[:], mu_c[:], rstd_c[:], OP.mult)
            nc.vector.tensor_scalar_mul(nmu_c[:], nmu_c[:], -1.0)
            # out = z*rstd - mu*rstd; chunks 2,3 on the DVE, 0,1 on the ACT
            for h in (2, 3):
                ts_, te_ = h * 4, (h + 1) * 4
                nc.vector.tensor_tensor(
                    o_v[:, ts_:te_, :], z_v[:, ts_:te_, :],
                    rstd_c[:, ts_:te_].unsqueeze(-1).broadcast_to([128, 4, DK]),
                    OP.mult)
                nc.vector.tensor_tensor(
                    o_v[:, ts_:te_, :], o_v[:, ts_:te_, :],
                    nmu_c[:, ts_:te_].unsqueeze(-1).broadcast_to([128, 4, DK]),
                    OP.add)
                nc.sync.dma_start(out_d[:, ts_ * DK:te_ * DK],
                                  o_sb[:, ts_ * DK:te_ * DK])
            for it in range(8):
                nc.scalar.activation(o_v[:, it, :], z_v[:, it, :], AF.Identity,
                                     bias=nmu_c[:, it:it + 1],
                                     scale=rstd_c[:, it:it + 1])
                if it % 4 == 3:
                    h = it // 4
                    nc.sync.dma_start(
                        out_d[:, h * 4 * DK:(h + 1) * 4 * DK],
                        o_sb[:, h * 4 * DK:(h + 1) * 4 * DK])

        big.release()
        cpool.release()

    if split:
        split_multiwaits(nc)
    return nc


_NC_CACHE = None


def _get_nc():
    global _NC_CACHE
    if _NC_CACHE is None:
        _NC_CACHE = build_nc()
    return _NC_CACHE


def _fourier_coeffs():
    m = 16384
    t = LPER * np.arange(m) / m
    tw = np.minimum(t, LPER - t)
    g = np.exp(np.exp(-tw ** 2) / 8.0) - 1.0
    c = np.fft.rfft(g) / m
    a_cos = np.concatenate([[1.0 + np.real(c[0])], 2 * np.real(c[1:13])])
    a_sin = 2 * np.real(c[1:12])
    return np.concatenate([a_cos, a_sin]).astype(np.float32)


def make_in_maps(x, Wv, bv, wq, wk, gamma, beta):
    import ml_dtypes
    bf = ml_dtypes.bfloat16
    x = np.asarray(x, np.float32)
    kfeat = np.concatenate([np.arange(13), np.arange(1, 12)]).astype(np.float64)
    phip = np.concatenate([0.25 * np.ones(13), np.zeros(11)])

    wvb = np.zeros((65, 66), np.float32)
    wvb[:64, :64] = np.asarray(Wv, np.float32).T
    wvb[64, :64] = np.asarray(bv, np.float32)
    wvb[64, 64] = 1.0
    wvb[:, 65] = wvb[:, :64].sum(1)

    blob = np.zeros((128, BLOB_W), np.float32)
    blob[0:65, _WVB0:_WVB0 + 66] = wvb
    blob[0:24, _ACOL] = _fourier_coeffs()

    def w2pair(w):
        full = np.concatenate(
            [np.outer(np.asarray(w, np.float64), kfeat / LPER),
             phip[None, :]], 0).astype(np.float32)
        hi = full.astype(bf)
        lo = (full - hi.astype(np.float32)).astype(bf)
        return hi, lo

    wkh, wkl = w2pair(wk)
    wqh, wql = w2pair(wq)
    csel = np.concatenate([np.ones(64, np.float32), [0.0]])[:, None]
    wf = np.concatenate([wkh, wkl, wqh, wql, csel.astype(bf)], 1).astype(bf)
    idb = np.eye(128, dtype=bf)

    ones = np.ones((N, 1), np.float32)
    in_maps = []
    for c in range(NCORES):
        b, qoff = c // 2, (c % 2) * NQ
        xr = np.concatenate([x[b, qoff:], x[b, :qoff]], axis=0) if qoff else x[b]
        xth = np.concatenate([xr.T, ones.T], 0).astype(bf)
        xaf = np.concatenate([xr, ones], 1)
        xa = xaf.astype(bf)                                  # (N, 65)
        xl = (xr[0:NQ] - xa[0:NQ, 0:DK].astype(np.float32)).astype(bf)
        # pre-tile to [p, tile, col] so device DMAs are contiguous
        xa_t = np.ascontiguousarray(
            xa.reshape(JT, 128, 65).transpose(1, 0, 2).reshape(128, JT * 65))
        xl_t = np.ascontiguousarray(
            xl.reshape(IT, 128, DK).transpose(1, 0, 2).reshape(128, IT * DK))
        in_maps.append({"xth": np.ascontiguousarray(xth),
                        "xa": xa_t, "xl": xl_t,
                        "wf": wf, "idb": idb, "blob": blob})
    return in_maps


def kernel(x, Wv, bv, wq, wk, gamma, beta, _trace=False, _trace_cores=None):
    nc = _get_nc()
    in_maps = make_in_maps(x, Wv, bv, wq, wk, gamma, beta)
    res = run_bass_kernel_spmd(nc, in_maps, core_ids=list(range(NCORES)),
                               trace=_trace, trace_cores=_trace_cores)
    out = np.empty((B, N, DK), np.float32)
    for c in range(NCORES):
        b, qoff = c // 2, (c % 2) * NQ
        oc = res.results[c]["out"].reshape(128, IT, DK).transpose(1, 0, 2)
        out[b, qoff:qoff + NQ] = oc.reshape(NQ, DK)
    # gamma/beta are ones/zeros in this problem's setup; apply on host if not.
    g = np.asarray(gamma, np.float32)
    bt = np.asarray(beta, np.float32)
    if not (np.all(g == 1.0) and np.all(bt == 0.0)):
        out = out * g + bt
    kernel._last_results = res
    return out
